# revision 33
# baseline (speedup 1.0000x reference)
"""Trainium2 Bass kernel for nn_AttLayer (4-head attention, softmax over queries).

Sharding: data-parallel over batch. 8 batch elements -> 8 NeuronCores, zero
collectives.

Key algebraic restructuring: with C=64 channels the attention is rank-65.
Folding the projections through the score/value contractions (bias rows
appended via the augmented-ones trick):

  R_h       = G_h^T-contracted input               G_h = Wk_aug_h @ Wq_aug_h^T
  scoresT_h = Xa^T R_h                             (= Xa^T G_h Xa, 65 x 65 G)
  es        = exp(SCALE * scoresT)                 row-sum den fused into the
                                                   exp activation (accum_out)
  xtr[j,c]  = XaT[j,c] / den[j]                    reciprocal folded into the
                                                   65-wide transposed input
  M2_h[c,i] = sum_j xtr[j,c] * es[j,i]             (65 x 1024)
  out2     += F_h^T @ M2_h                         F_h = Wv_aug_h @ Wout_h
  out       = out2 + b_out + x

G_h and F_h are computed on the host in f32 (exact). Everything on-chip is
bf16 matmuls with f32 PSUM accumulation; the exp/normalize core is the
critical path (ScalarEngine), so all other work is software-pipelined into
the per-j-tile chain steps of neighboring heads.
"""

import numpy as np
import ml_dtypes

import concourse.tile as tile
from concourse import bacc, mybir
from concourse.bass_utils import run_bass_kernel_spmd

NH = 4          # heads
D = 640         # per-head dim
C = 64          # channels
CA = C + 1      # augmented (ones row)
CP = 80         # CA padded so the DoubleRow k-subtile byte stride is 16-aligned
ES_SCALE = 64.0  # fp8 headroom for the normalized attention weights
SEQ = 1024      # 32*32
SCALE = float(D) ** -0.5
N_CORES = 8
FP = mybir.dt.float32
BF = mybir.dt.bfloat16
F8 = mybir.dt.float8e4

JT = SEQ // 128     # 8 j-tiles (128 keys each)
IC = SEQ // 512     # 2 i-chunks (512 queries each)

AF = mybir.ActivationFunctionType
ALU = mybir.AluOpType


def _build():
    nc = bacc.Bacc(None, target_bir_lowering=False)
    # packed input blobs: one DMA each (descriptor setup dominates small DMAs)
    W_QA = SEQ + NH * CA + NH * C          # xa | gt | ff   on rows 0..64
    qa = nc.declare_dram_parameter("qa", [CA, W_QA], BF, isOutput=False)
    xt = nc.declare_dram_parameter("xt", [128, JT * CP], BF, isOutput=False)
    kb = nc.declare_dram_parameter("kb", [128, NH * JT], FP, isOutput=False)
    xr = nc.declare_dram_parameter("xr", [C, SEQ + 1], FP, isOutput=False)
    out = nc.declare_dram_parameter("out", [C, SEQ], FP, isOutput=True)

    with tile.TileContext(nc) as tc:
        with (
            tc.tile_pool(name="consts", bufs=1) as consts,
            tc.tile_pool(name="hpool", bufs=4) as hpool,
            tc.tile_pool(name="sc", bufs=2, space="PSUM") as sc_psum,
            tc.tile_pool(name="pm", bufs=2, space="PSUM") as pm_psum,
        ):
            kb_sb = consts.tile([128, NH * JT], FP)
            nc.sync.dma_start(out=kb_sb[:], in_=kb[:, :])
            qa_sb = consts.tile([CA, W_QA], BF)
            # weights chunk first (gates R), then the two xa halves, each on
            # its own DMA queue
            nc.sync.dma_start(out=qa_sb[:, SEQ:], in_=qa[:, SEQ:])
            for ic in range(IC):
                nc.sync.dma_start(
                    out=qa_sb[:, ic * 512:(ic + 1) * 512],
                    in_=qa[:, ic * 512:(ic + 1) * 512],
                )
            xtb_sb = consts.tile([128, JT * CP], BF)
            xr_sb = consts.tile([C, SEQ + 1], FP)
            xa_sb = qa_sb[:, 0:SEQ]

            def gt_view(h):
                return qa_sb[:, SEQ + h * CA: SEQ + (h + 1) * CA]

            def ff_view(h):
                return qa_sb[:, SEQ + NH * CA + h * C: SEQ + NH * CA + (h + 1) * C]

            def xt_view(jt):
                return xtb_sb[:, jt * CP:(jt + 1) * CP]

            xf_sb = xr_sb[:, 0:SEQ]
            bo_sb = xr_sb[:, SEQ:SEQ + 1]
            out_sb = consts.tile([C, SEQ], FP)
            o2acc = consts.tile([C, SEQ], FP)

            def emit_late_dmas():
                nc.sync.dma_start(out=xtb_sb[:], in_=xt[:, :])
                nc.sync.dma_start(out=xr_sb[:], in_=xr[:, :])

            def emit_R_ic(h, ic, state):
                if ic == 0:
                    state = (
                        hpool.tile([CA, SEQ], BF, tag="R", name=f"R_{h}"),
                        pm_psum.tile([CA, SEQ], FP, tag="pm", name=f"rp_{h}"),
                    )
                R_sb, rps = state
                nc.tensor.matmul(
                    rps[:, ic * 512:(ic + 1) * 512],
                    lhsT=gt_view(h),
                    rhs=xa_sb[:, ic * 512:(ic + 1) * 512],
                    start=True, stop=True,
                )
                nc.vector.tensor_copy(
                    out=R_sb[:, ic * 512:(ic + 1) * 512],
                    in_=rps[:, ic * 512:(ic + 1) * 512],
                )
                return state

            def emit_R(h):
                state = emit_R_ic(h, 0, None)
                state = emit_R_ic(h, 1, state)
                return state[0]

            def emit_M2_mms(mps, xtr, es, jp):
                # fp8 DoubleRow over the (jp, jp+1) j-tile pair
                for ic in range(IC):
                    nc.tensor.matmul(
                        mps[:, ic * 512:(ic + 1) * 512],
                        lhsT=xtr[:, jp:jp + 2, :],
                        rhs=es[:, jp:jp + 2, ic * 512:(ic + 1) * 512],
                        start=(jp == 0), stop=(jp == JT - 2),
                        perf_mode=mybir.MatmulPerfMode.DoubleRow,
                    )

            def emit_m2_conv(ph, pmps):
                pm2 = hpool.tile([CA, SEQ], BF, tag="m2", name=f"m2_{ph}")
                for ic in range(IC):
                    nc.vector.tensor_copy(
                        out=pm2[:, ic * 512:(ic + 1) * 512],
                        in_=pmps[:CA, ic * 512:(ic + 1) * 512],
                    )
                return pm2

            def emit_out2(h, m2):
                o2p = pm_psum.tile([CA, SEQ], FP, tag="pm", name=f"o2_{h}")
                for ic in range(IC):
                    nc.tensor.matmul(
                        o2p[:C, ic * 512:(ic + 1) * 512],
                        lhsT=ff_view(h),
                        rhs=m2[:, ic * 512:(ic + 1) * 512],
                        start=True, stop=True,
                    )
                if h == 0:
                    nc.vector.tensor_copy(out=o2acc[:], in_=o2p[:C, :])
                elif h < NH - 1:
                    nc.vector.tensor_add(out=o2acc[:], in0=o2acc[:], in1=o2p[:C, :])
                else:
                    # final head: o2acc already holds heads 0-2 plus residual
                    for ic in range(IC):
                        sl = slice(ic * 512, (ic + 1) * 512)
                        nc.scalar.activation(
                            out=out_sb[:, sl],
                            in_=o2p[:C, sl],
                            func=AF.Identity,
                            bias=bo_sb[:],
                            scale=1.0,
                        )
                        nc.vector.tensor_add(
                            out=out_sb[:, sl], in0=out_sb[:, sl], in1=o2acc[:, sl],
                        )
                        for q in range(2):
                            qsl = slice(ic * 512 + q * 256, ic * 512 + (q + 1) * 256)
                            nc.sync.dma_start(out=out[:, qsl], in_=out_sb[:, qsl])

            R_cur = emit_R(0)
            emit_late_dmas()
            R_nxt = None
            prev = None   # (h, es, xtr, mps) of the previous head
            for h in range(NH):
                R_sb = R_cur
                last = h == NH - 1
                es = hpool.tile([128, JT, SEQ], F8, tag="es", name=f"es_{h}")
                xtr = hpool.tile([128, JT, CP], F8, tag="xtr", name=f"xtr_{h}")
                den = hpool.tile([128, JT], FP, tag="den", name=f"den_{h}")
                rec = hpool.tile([128, JT], FP, tag="rec", name=f"rec_{h}")
                own_mps = (
                    pm_psum.tile([CP, SEQ], FP, tag="pm", name="mp_last")
                    if last else None
                )

                for jt in range(JT):
                    pst = sc_psum.tile([128, SEQ], FP, tag="sc", name=f"sc_{h}_{jt}")
                    for ic in range(IC):
                        nc.tensor.matmul(
                            pst[:, ic * 512:(ic + 1) * 512],
                            lhsT=xa_sb[:, jt * 128:(jt + 1) * 128],
                            rhs=R_sb[:, ic * 512:(ic + 1) * 512],
                            start=True, stop=True,
                        )
                    nc.scalar.activation(
                        out=es[:, jt, :],
                        in_=pst[:],
                        func=AF.Exp,
                        scale=SCALE,
                        bias=kb_sb[:, h * JT + jt: h * JT + jt + 1],
                        accum_out=den[:, jt:jt + 1],
                    )
                    nc.vector.reciprocal(out=rec[:, jt:jt + 1], in_=den[:, jt:jt + 1])
                    nc.vector.tensor_scalar_mul(
                        xtr[:, jt, :], xt_view(jt), rec[:, jt:jt + 1],
                    )

                    # ---- pipelined injections (<=2 matmuls per chain step)
                    if prev is not None:
                        ph, pes, pxtr, pmps = prev
                        if jt <= 3:
                            emit_M2_mms(pmps, pxtr, pes, 2 * jt)
                        if jt == JT - 1:
                            pm2 = emit_m2_conv(ph, pmps)
                            emit_out2(ph, pm2)
                            prev = None
                    if jt == 2 and h + 1 < NH:
                        R_state = emit_R_ic(h + 1, 0, None)
                    if jt == 3 and h + 1 < NH:
                        R_nxt = emit_R_ic(h + 1, 1, R_state)[0]
                    if last and jt % 2 == 1 and jt < 7:
                        emit_M2_mms(own_mps, xtr, es, jt - 1)
                    if last and jt == 7:
                        nc.vector.tensor_add(
                            out=o2acc[:], in0=o2acc[:], in1=xf_sb[:],
                        )

                if not last:
                    mps = pm_psum.tile([CP, SEQ], FP, tag="pm", name=f"mp_{h}")
                    prev = (h, es, xtr, mps)
                R_cur = R_nxt

            # drain the last head's M2 tail (j-tile 7) and final output,
            # fully per-i-chunk so DVE/PE/ACT/DMA overlap
            emit_M2_mms(own_mps, xtr, es, 6)
            pm2 = hpool.tile([CA, SEQ], BF, tag="m2", name="m2_last")
            o2p = pm_psum.tile([CA, SEQ], FP, tag="pm", name="o2_last")
            for ic in range(IC):
                sl = slice(ic * 512, (ic + 1) * 512)
                nc.vector.tensor_copy(out=pm2[:, sl], in_=own_mps[:CA, sl])
                nc.tensor.matmul(
                    o2p[:C, sl],
                    lhsT=ff_view(NH - 1),
                    rhs=pm2[:, sl],
                    start=True, stop=True,
                )
                nc.scalar.activation(
                    out=out_sb[:, sl],
                    in_=o2p[:C, sl],
                    func=AF.Identity,
                    bias=bo_sb[:],
                    scale=1.0,
                )
                nc.vector.tensor_add(
                    out=out_sb[:, sl], in0=out_sb[:, sl], in1=o2acc[:, sl],
                )
                for q in range(2):
                    qsl = slice(ic * 512 + q * 256, ic * 512 + (q + 1) * 256)
                    nc.sync.dma_start(out=out[:, qsl], in_=out_sb[:, qsl])

    nc.compile()
    return nc


_CACHE: dict = {}


def _get_nc():
    if "nc" not in _CACHE:
        _CACHE["nc"] = _build()
    return _CACHE["nc"]


def _prep_in_maps(x, W_proj, b_proj, W_out, b_out):
    bf = ml_dtypes.bfloat16
    x = np.ascontiguousarray(np.asarray(x, dtype=np.float32))
    W_proj = np.asarray(W_proj, dtype=np.float32)
    b_proj = np.asarray(b_proj, dtype=np.float32)
    W_out = np.asarray(W_out, dtype=np.float32)
    b_out = np.asarray(b_out, dtype=np.float32)

    x2 = x.reshape(N_CORES, C, SEQ)
    W_QA = SEQ + NH * CA + NH * C

    # augmented per-head projection blocks [65, 640]
    Wa = np.concatenate([W_proj, b_proj[None, :]], axis=0)  # [65, 7680]
    gt = np.empty((CA, NH, CA), dtype=np.float32)
    ffm = np.empty((CA, NH, C), dtype=np.float32)
    for h in range(NH):
        q0 = h * 3 * D
        Wq = Wa[:, q0:q0 + D]            # [65, 640]
        Wk = Wa[:, q0 + D:q0 + 2 * D]
        Wv = Wa[:, q0 + 2 * D:q0 + 3 * D]
        G = Wk @ Wq.T                    # [65, 65]; scoresT = Xa^T G Xa
        gt[:, h, :] = G.T                # lhsT[c', c] = G[c, c']
        ffm[:, h, :] = Wv @ W_out[h * D:(h + 1) * D, :]   # [65, 64]

    # blob 1: xa | gt | ff  on 65 partitions
    qa_all = np.empty((N_CORES, CA, W_QA), dtype=bf)
    qa_all[:, :C, :SEQ] = x2.astype(bf)
    qa_all[:, C, :SEQ] = np.float32(1.0)
    qa_all[:, :, SEQ:SEQ + NH * CA] = gt.reshape(CA, NH * CA).astype(bf)[None]
    qa_all[:, :, SEQ + NH * CA:] = ffm.reshape(CA, NH * C).astype(bf)[None]

    # blob 2: XaT [p, jt*CP+c'] = x[c', jt*128+p], ones at c'=64, zero pad
    xt_all = np.zeros((N_CORES, 128, JT, CP), dtype=bf)
    xtt = x2.transpose(0, 2, 1).reshape(N_CORES, JT, 128, C)  # [b, jt, p, c]
    xt_all[:, :, :, :C] = xtt.transpose(0, 2, 1, 3).astype(bf)
    xt_all[:, :, :, C] = np.float32(1.0)
    xt_all = xt_all.reshape(N_CORES, 128, JT * CP)

    # per-row exp bias: ln(ES_SCALE / den_est[j]) with den_est from the
    # lognormal moments of each score row (exact row mean/variance via G and
    # the input first/second moments). The shift cancels exactly in M2; it
    # only has to keep es = ES_SCALE*exp(s)/den_est and xtr = xt*den_est/
    # (ES_SCALE*den) inside fp8e4 range, so +-1 octave of estimate error is
    # harmless.
    kb_all = np.empty((N_CORES, 128, NH * JT), dtype=np.float32)
    for i in range(N_CORES):
        Xa = np.concatenate([x2[i], np.ones((1, SEQ), np.float32)], axis=0)
        m = Xa.mean(axis=1)
        Cov = (Xa @ Xa.T) / SEQ - np.outer(m, m)
        for h in range(NH):
            Gh = gt[:, h, :].T  # undo the transpose: Gh[c, c']
            mu = SCALE * (Xa.T @ (Gh @ m))                      # [1024]
            B = Gh @ Cov @ Gh.T
            v = (SCALE ** 2) * np.einsum("cj,cj->j", Xa, B @ Xa)
            log_den_est = np.log(float(SEQ)) + mu + 0.5 * v
            bias = np.log(ES_SCALE) - log_den_est
            kb_all[i, :, h * JT:(h + 1) * JT] = bias.reshape(JT, 128).T

    # blob 3: xf | b_out in f32
    xr_all = np.empty((N_CORES, C, SEQ + 1), dtype=np.float32)
    xr_all[:, :, :SEQ] = x2
    xr_all[:, :, SEQ] = b_out[None, :]

    return [
        {
            "qa": np.ascontiguousarray(qa_all[i]),
            "xt": np.ascontiguousarray(xt_all[i]),
            "kb": np.ascontiguousarray(kb_all[i]),
            "xr": np.ascontiguousarray(xr_all[i]),
        }
        for i in range(N_CORES)
    ]


def run(x, t, W_proj, b_proj, W_out, b_out, trace=False, **trace_kwargs):
    in_maps = _prep_in_maps(x, W_proj, b_proj, W_out, b_out)
    res = run_bass_kernel_spmd(
        _get_nc(), in_maps, core_ids=list(range(N_CORES)),
        trace=trace, **trace_kwargs,
    )
    out = np.stack([res.results[i]["out"] for i in range(N_CORES)])
    return out.reshape(N_CORES, C, 32, 32), res


def kernel(x, t=None, W_proj=None, b_proj=None, W_out=None, b_out=None):
    out, _ = run(x, t, W_proj, b_proj, W_out, b_out, trace=False)
    return out


# revision 34
# speedup vs baseline: 1.1400x; 1.1400x over previous
"""Trainium2 Bass kernel for nn_AttLayer (4-head attention, softmax over queries).

Sharding: data-parallel over batch. 8 batch elements -> 8 NeuronCores, zero
collectives.

Key algebraic restructuring: with C=64 channels the attention is rank-65.
Folding the projections through the score/value contractions (bias rows
appended via the augmented-ones trick):

  R_h       = G_h^T-contracted input               G_h = Wk_aug_h @ Wq_aug_h^T
  scoresT_h = Xa^T R_h                             (= Xa^T G_h Xa, 65 x 65 G)
  es        = exp(SCALE * scoresT)                 row-sum den fused into the
                                                   exp activation (accum_out)
  xtr[j,c]  = XaT[j,c] / den[j]                    reciprocal folded into the
                                                   65-wide transposed input
  M2_h[c,i] = sum_j xtr[j,c] * es[j,i]             (65 x 1024)
  out2     += F_h^T @ M2_h                         F_h = Wv_aug_h @ Wout_h
  out       = out2 + b_out + x

G_h and F_h are computed on the host in f32 (exact). Everything on-chip is
bf16 matmuls with f32 PSUM accumulation; the exp/normalize core is the
critical path (ScalarEngine), so all other work is software-pipelined into
the per-j-tile chain steps of neighboring heads.
"""

import numpy as np
import ml_dtypes

import concourse.tile as tile
from concourse import bacc, mybir
from concourse.bass_utils import run_bass_kernel_spmd

NH = 4          # heads
D = 640         # per-head dim
C = 64          # channels
CA = C + 1      # augmented (ones row)
SEQ = 1024      # 32*32
SCALE = float(D) ** -0.5
N_CORES = 8
FP = mybir.dt.float32
BF = mybir.dt.bfloat16

JT = SEQ // 128     # 8 j-tiles (128 keys each)
IC = SEQ // 512     # 2 i-chunks (512 queries each)

AF = mybir.ActivationFunctionType
ALU = mybir.AluOpType


def _build():
    nc = bacc.Bacc(None, target_bir_lowering=False)
    # packed input blobs: one DMA each (descriptor setup dominates small DMAs)
    W_QA = SEQ + NH * CA + NH * C          # xa | gt | ff   on rows 0..64
    qa = nc.declare_dram_parameter("qa", [CA, W_QA], BF, isOutput=False)
    xt = nc.declare_dram_parameter("xt", [128, JT * CA], BF, isOutput=False)
    xr = nc.declare_dram_parameter("xr", [C, SEQ + 1], FP, isOutput=False)
    out = nc.declare_dram_parameter("out", [C, SEQ], FP, isOutput=True)

    with tile.TileContext(nc) as tc:
        with (
            tc.tile_pool(name="consts", bufs=1) as consts,
            tc.tile_pool(name="hpool", bufs=4) as hpool,
            tc.tile_pool(name="sc", bufs=2, space="PSUM") as sc_psum,
            tc.tile_pool(name="pm", bufs=2, space="PSUM") as pm_psum,
        ):
            qa_sb = consts.tile([CA, W_QA], BF)
            # weights chunk first (gates R), then the two xa halves, each on
            # its own DMA queue
            nc.sync.dma_start(out=qa_sb[:, SEQ:], in_=qa[:, SEQ:])
            for ic in range(IC):
                nc.sync.dma_start(
                    out=qa_sb[:, ic * 512:(ic + 1) * 512],
                    in_=qa[:, ic * 512:(ic + 1) * 512],
                )
            xtb_sb = consts.tile([128, JT * CA], BF)
            xr_sb = consts.tile([C, SEQ + 1], FP)
            xa_sb = qa_sb[:, 0:SEQ]

            def gt_view(h):
                return qa_sb[:, SEQ + h * CA: SEQ + (h + 1) * CA]

            def ff_view(h):
                return qa_sb[:, SEQ + NH * CA + h * C: SEQ + NH * CA + (h + 1) * C]

            def xt_view(jt):
                return xtb_sb[:, jt * CA:(jt + 1) * CA]

            xf_sb = xr_sb[:, 0:SEQ]
            bo_sb = xr_sb[:, SEQ:SEQ + 1]
            out_sb = consts.tile([C, SEQ], FP)
            o2acc = consts.tile([C, SEQ], FP)

            def emit_late_dmas():
                nc.sync.dma_start(out=xtb_sb[:], in_=xt[:, :])
                nc.sync.dma_start(out=xr_sb[:], in_=xr[:, :])

            def emit_R_ic(h, ic, state):
                if ic == 0:
                    state = (
                        hpool.tile([CA, SEQ], BF, tag="R", name=f"R_{h}"),
                        pm_psum.tile([CA, SEQ], FP, tag="pm", name=f"rp_{h}"),
                    )
                R_sb, rps = state
                nc.tensor.matmul(
                    rps[:, ic * 512:(ic + 1) * 512],
                    lhsT=gt_view(h),
                    rhs=xa_sb[:, ic * 512:(ic + 1) * 512],
                    start=True, stop=True,
                )
                nc.vector.tensor_copy(
                    out=R_sb[:, ic * 512:(ic + 1) * 512],
                    in_=rps[:, ic * 512:(ic + 1) * 512],
                )
                return state

            def emit_R(h):
                state = emit_R_ic(h, 0, None)
                state = emit_R_ic(h, 1, state)
                return state[0]

            def emit_M2_mms(mps, xtr, es, jt):
                for ic in range(IC):
                    nc.tensor.matmul(
                        mps[:, ic * 512:(ic + 1) * 512],
                        lhsT=xtr[:, jt, :],
                        rhs=es[:, jt, ic * 512:(ic + 1) * 512],
                        start=(jt == 0), stop=(jt == JT - 1),
                    )

            def emit_m2_conv(ph, pmps):
                pm2 = hpool.tile([CA, SEQ], BF, tag="m2", name=f"m2_{ph}")
                for ic in range(IC):
                    nc.vector.tensor_copy(
                        out=pm2[:, ic * 512:(ic + 1) * 512],
                        in_=pmps[:, ic * 512:(ic + 1) * 512],
                    )
                return pm2

            def emit_out2(h, m2):
                o2p = pm_psum.tile([CA, SEQ], FP, tag="pm", name=f"o2_{h}")
                for ic in range(IC):
                    nc.tensor.matmul(
                        o2p[:C, ic * 512:(ic + 1) * 512],
                        lhsT=ff_view(h),
                        rhs=m2[:, ic * 512:(ic + 1) * 512],
                        start=True, stop=True,
                    )
                if h == 0:
                    nc.vector.tensor_copy(out=o2acc[:], in_=o2p[:C, :])
                elif h < NH - 1:
                    nc.vector.tensor_add(out=o2acc[:], in0=o2acc[:], in1=o2p[:C, :])
                else:
                    # final head: o2acc already holds heads 0-2 plus residual
                    for ic in range(IC):
                        sl = slice(ic * 512, (ic + 1) * 512)
                        nc.scalar.activation(
                            out=out_sb[:, sl],
                            in_=o2p[:C, sl],
                            func=AF.Identity,
                            bias=bo_sb[:],
                            scale=1.0,
                        )
                        nc.vector.tensor_add(
                            out=out_sb[:, sl], in0=out_sb[:, sl], in1=o2acc[:, sl],
                        )
                        for q in range(2):
                            qsl = slice(ic * 512 + q * 256, ic * 512 + (q + 1) * 256)
                            nc.sync.dma_start(out=out[:, qsl], in_=out_sb[:, qsl])

            R_cur = emit_R(0)
            emit_late_dmas()
            R_nxt = None
            prev = None   # (h, es, xtr, mps) of the previous head
            for h in range(NH):
                R_sb = R_cur
                last = h == NH - 1
                es = hpool.tile([128, JT, SEQ], BF, tag="es", name=f"es_{h}")
                xtr = hpool.tile([128, JT, CA], BF, tag="xtr", name=f"xtr_{h}")
                den = hpool.tile([128, JT], FP, tag="den", name=f"den_{h}")
                rec = hpool.tile([128, JT], FP, tag="rec", name=f"rec_{h}")
                own_mps = (
                    pm_psum.tile([CA, SEQ], FP, tag="pm", name="mp_last")
                    if last else None
                )

                for jt in range(JT):
                    pst = sc_psum.tile([128, SEQ], FP, tag="sc", name=f"sc_{h}_{jt}")
                    for ic in range(IC):
                        nc.tensor.matmul(
                            pst[:, ic * 512:(ic + 1) * 512],
                            lhsT=xa_sb[:, jt * 128:(jt + 1) * 128],
                            rhs=R_sb[:, ic * 512:(ic + 1) * 512],
                            start=True, stop=True,
                        )
                    nc.scalar.activation(
                        out=es[:, jt, :],
                        in_=pst[:],
                        func=AF.Exp,
                        scale=SCALE,
                        accum_out=den[:, jt:jt + 1],
                    )
                    nc.vector.reciprocal(out=rec[:, jt:jt + 1], in_=den[:, jt:jt + 1])
                    nc.vector.tensor_scalar_mul(
                        xtr[:, jt, :], xt_view(jt), rec[:, jt:jt + 1],
                    )

                    # ---- pipelined injections (<=2 matmuls per chain step)
                    if prev is not None:
                        ph, pes, pxtr, pmps = prev
                        emit_M2_mms(pmps, pxtr, pes, jt)
                        if jt == JT - 1:
                            pm2 = emit_m2_conv(ph, pmps)
                            emit_out2(ph, pm2)
                            prev = None
                    if jt == 2 and h + 1 < NH:
                        R_state = emit_R_ic(h + 1, 0, None)
                    if jt == 3 and h + 1 < NH:
                        R_nxt = emit_R_ic(h + 1, 1, R_state)[0]
                    if last and jt >= 1:
                        emit_M2_mms(own_mps, xtr, es, jt - 1)
                    if last and jt == 7:
                        nc.vector.tensor_add(
                            out=o2acc[:], in0=o2acc[:], in1=xf_sb[:],
                        )

                if not last:
                    mps = pm_psum.tile([CA, SEQ], FP, tag="pm", name=f"mp_{h}")
                    prev = (h, es, xtr, mps)
                R_cur = R_nxt

            # drain the last head's M2 tail (j-tile 7) and final output,
            # fully per-i-chunk so DVE/PE/ACT/DMA overlap
            emit_M2_mms(own_mps, xtr, es, 7)
            pm2 = hpool.tile([CA, SEQ], BF, tag="m2", name="m2_last")
            o2p = pm_psum.tile([CA, SEQ], FP, tag="pm", name="o2_last")
            for ic in range(IC):
                sl = slice(ic * 512, (ic + 1) * 512)
                nc.vector.tensor_copy(out=pm2[:, sl], in_=own_mps[:, sl])
                nc.tensor.matmul(
                    o2p[:C, sl],
                    lhsT=ff_view(NH - 1),
                    rhs=pm2[:, sl],
                    start=True, stop=True,
                )
                nc.scalar.activation(
                    out=out_sb[:, sl],
                    in_=o2p[:C, sl],
                    func=AF.Identity,
                    bias=bo_sb[:],
                    scale=1.0,
                )
                nc.vector.tensor_add(
                    out=out_sb[:, sl], in0=out_sb[:, sl], in1=o2acc[:, sl],
                )
                for q in range(2):
                    qsl = slice(ic * 512 + q * 256, ic * 512 + (q + 1) * 256)
                    nc.sync.dma_start(out=out[:, qsl], in_=out_sb[:, qsl])

    nc.compile()
    return nc


_CACHE: dict = {}


def _get_nc():
    if "nc" not in _CACHE:
        _CACHE["nc"] = _build()
    return _CACHE["nc"]


def _prep_in_maps(x, W_proj, b_proj, W_out, b_out):
    bf = ml_dtypes.bfloat16
    x = np.ascontiguousarray(np.asarray(x, dtype=np.float32))
    W_proj = np.asarray(W_proj, dtype=np.float32)
    b_proj = np.asarray(b_proj, dtype=np.float32)
    W_out = np.asarray(W_out, dtype=np.float32)
    b_out = np.asarray(b_out, dtype=np.float32)

    x2 = x.reshape(N_CORES, C, SEQ)
    W_QA = SEQ + NH * CA + NH * C

    # augmented per-head projection blocks [65, 640]
    Wa = np.concatenate([W_proj, b_proj[None, :]], axis=0)  # [65, 7680]
    gt = np.empty((CA, NH, CA), dtype=np.float32)
    ffm = np.empty((CA, NH, C), dtype=np.float32)
    for h in range(NH):
        q0 = h * 3 * D
        Wq = Wa[:, q0:q0 + D]            # [65, 640]
        Wk = Wa[:, q0 + D:q0 + 2 * D]
        Wv = Wa[:, q0 + 2 * D:q0 + 3 * D]
        G = Wk @ Wq.T                    # [65, 65]; scoresT = Xa^T G Xa
        gt[:, h, :] = G.T                # lhsT[c', c] = G[c, c']
        ffm[:, h, :] = Wv @ W_out[h * D:(h + 1) * D, :]   # [65, 64]

    # blob 1: xa | gt | ff  on 65 partitions
    qa_all = np.empty((N_CORES, CA, W_QA), dtype=bf)
    qa_all[:, :C, :SEQ] = x2.astype(bf)
    qa_all[:, C, :SEQ] = np.float32(1.0)
    qa_all[:, :, SEQ:SEQ + NH * CA] = gt.reshape(CA, NH * CA).astype(bf)[None]
    qa_all[:, :, SEQ + NH * CA:] = ffm.reshape(CA, NH * C).astype(bf)[None]

    # blob 2: XaT [p, jt*65+c'] = x[c', jt*128+p], ones at c'=64
    xt_all = np.empty((N_CORES, 128, JT, CA), dtype=bf)
    xtt = x2.transpose(0, 2, 1).reshape(N_CORES, JT, 128, C)  # [b, jt, p, c]
    xt_all[:, :, :, :C] = xtt.transpose(0, 2, 1, 3).astype(bf)
    xt_all[:, :, :, C] = np.float32(1.0)
    xt_all = xt_all.reshape(N_CORES, 128, JT * CA)

    # blob 3: xf | b_out in f32
    xr_all = np.empty((N_CORES, C, SEQ + 1), dtype=np.float32)
    xr_all[:, :, :SEQ] = x2
    xr_all[:, :, SEQ] = b_out[None, :]

    return [
        {
            "qa": np.ascontiguousarray(qa_all[i]),
            "xt": np.ascontiguousarray(xt_all[i]),
            "xr": np.ascontiguousarray(xr_all[i]),
        }
        for i in range(N_CORES)
    ]


def run(x, t, W_proj, b_proj, W_out, b_out, trace=False, **trace_kwargs):
    in_maps = _prep_in_maps(x, W_proj, b_proj, W_out, b_out)
    res = run_bass_kernel_spmd(
        _get_nc(), in_maps, core_ids=list(range(N_CORES)),
        trace=trace, **trace_kwargs,
    )
    out = np.stack([res.results[i]["out"] for i in range(N_CORES)])
    return out.reshape(N_CORES, C, 32, 32), res


def kernel(x, t=None, W_proj=None, b_proj=None, W_out=None, b_out=None):
    out, _ = run(x, t, W_proj, b_proj, W_out, b_out, trace=False)
    return out
